# revision 4
# baseline (speedup 1.0000x reference)
"""Bass/Trainium2 kernel for nn_DiscriminativeCorrelationFilter.

Math
----
Reference computes, per batch b:
  sp = BN(W @ xs_b), tp = BN(W @ xt_b)        (1x1 conv 768->768 + eval-mode BN)
  label from mask centroid (Gaussian); f_0 = f_init; 5 iterations of a
  hinge-gradient update whose gradient is a per-batch SCALAR; then
  out_b = f_5 . sp.
Because BN(W@x) = inv_std .* (W@x) + cvec and f_t stays in
span{f_init, ones}, every channel contraction collapses onto two fixed
vectors p = W^T (f_init .* inv_std), q = W^T inv_std with scalars
k1 = f_init.cvec, k2 = sum(cvec):
    f_t . BN(W@x) = a_t (p^T x + k1) + c_t (q^T x + k2),  a_t = rho^t.
Device work = stream the features through [p;q] projections + a tiny
5-step scalar recurrence; out = a5*(P + ctil5*Q) + a5*(k1 + k2*ctil5)
rides the host unshard step (65 KFLOP total).

Performance structure (v3; the kernel is feature-DMA bound):
  * the PE multiplies fp16 stationary x fp8 moving exactly (verified
    on HW), so all stationaries are fp16 and only the feature STREAM
    is quantized.
  * search features: channels permuted by projection-weight energy;
    top 128 channels stream fp16, remaining 640 as five e3m4 chunks
    (3.67 MB/core). Search PSUM rows per batch: [P, Q] at partition
    32b via col-group packing; one fp16 tile export; host combine.
  * target features all e3m4 (0.77 MB/core), 3 chunk-pair DMAs so
    projections start on first arrival. U = p^T xt and S = q^T xt are
    two M=1 passes into col-group 32b of two PSUM tiles, so U_b and
    S_b land on the SAME partition row 32b -- the whole recurrence
    runs on 128-partition tiles (live rows 32b) with label/glm
    constants DMA'd straight to rows 32b, reading U/S from PSUM.
    No cross-partition shuffles anywhere.
  * every feature tensor is host-packed SBUF-shaped: each DMA is one
    fully contiguous multi-KB segment per partition, issued on the
    sync HWDGE ring in consumption order (xt pairs, xs16, xs8 c1..5).
End-to-end quantization error (deterministic, fixed seed): ~1.1e-2
absmax-relative vs the 2e-2 gate.

Sharding: data-parallel over batch, 4 batches per core on 8 cores.
"""

import time

import numpy as np
from contextlib import ExitStack

import concourse.bacc as bacc
import concourse.mybir as mybir
import concourse.tile as tile
from concourse.bass_utils import run_bass_kernel_spmd
import ml_dtypes

# ---------------- problem constants (hardcoded; kernel.py must be standalone)
B = 32
D = 768
HS = WS = 32
HT = WT = 16
NS = HS * WS      # 1024
NT = HT * WT      # 256
NCORES = 8
BPC = B // NCORES  # 4
KC = D // 128      # 6
KHI = 128          # channels kept in fp16 (chunk 0 after permutation)
KC8 = (D - KHI) // 128   # 5 fp8 chunks

LR = 0.1
LAM = 0.01
SIGMA = 2.0
NIT = 5
BN_EPS = 1e-5
RHO = 1.0 - LR * LAM
A5 = RHO ** NIT

F32 = mybir.dt.float32
F16 = mybir.dt.float16
F8 = mybir.dt.float8e3       # e3m4
NP_F8 = ml_dtypes.float8_e3m4

_CACHE = {}


def build():
    nc = bacc.Bacc()
    xt8 = nc.dram_tensor("xt8", (128, KC, BPC * NT), F8, kind="ExternalInput")
    xs16 = nc.dram_tensor("xs16", (128, BPC * NS), F16, kind="ExternalInput")
    xs8 = nc.dram_tensor("xs8", (128, KC8, BPC * NS), F8, kind="ExternalInput")
    # cols 0..11: natural-order [p_k, q_k] pairs (target stage)
    # cols 12..23: permuted [p_c, q_c] pairs (search stage, chunks 0..5)
    pqw = nc.dram_tensor("pqw", (128, 24), F16, kind="ExternalInput")
    cstd = nc.dram_tensor("cst", (BPC, 6 * NT + 4), F32, kind="ExternalInput")
    pqo = nc.dram_tensor("pqo", (128, 2, 512), F16, kind="ExternalOutput")
    cto = nc.dram_tensor("cto", (BPC, 1), F32, kind="ExternalOutput")

    AL = mybir.AluOpType
    CW = 6 * NT + 4

    with tile.TileContext(nc) as tc, ExitStack() as ctx:
        const = ctx.enter_context(tc.tile_pool(name="const", bufs=1))
        feats = ctx.enter_context(tc.tile_pool(name="feats", bufs=1))
        work = ctx.enter_context(tc.tile_pool(name="work", bufs=1))
        psum = ctx.enter_context(tc.tile_pool(name="psum", bufs=8, space="PSUM"))

        # ---- constants (scalar/ACT HWDGE ring): pqw first (first consumer),
        # then per-batch constants straight to partition rows 32b
        pqw_sb = const.tile([128, 24], F16, tag="pqw")
        nc.scalar.dma_start(pqw_sb[:, :], pqw[:, :])
        cstB = const.tile([128, CW], F32, tag="cstB")
        for b in range(BPC):
            nc.scalar.dma_start(cstB[32 * b:32 * b + 1, :], cstd[b:b + 1, :])
        labB = cstB[:, 0:NT]
        glmB = [cstB[:, (1 + t) * NT:(2 + t) * NT] for t in range(NIT)]
        karB = cstB[:, 6 * NT:6 * NT + 4]

        # ---- feature loads (sync HWDGE ring) in consumption order
        xtj = []
        for j in range(3):
            t = feats.tile([128, 2, BPC * NT], F8, tag=f"xt{j}", name=f"xt{j}")
            nc.sync.dma_start(t[:, :, :], xt8[:, 2 * j:2 * j + 2, :])
            xtj.append(t)
        xs16_sb = feats.tile([128, BPC * NS], F16, tag="xs16", name="xs16")
        nc.sync.dma_start(xs16_sb[:, :], xs16[:, :])
        xs8_sb = []
        for c in range(KC8):
            t = feats.tile([128, BPC * NS], F8, tag=f"xs8_{c}", name=f"xs8_{c}")
            nc.sync.dma_start(t[:, :], xs8[:, c, :])
            xs8_sb.append(t)

        # ---- target stage: U_b -> psU row 32b, S_b -> psS row 32b
        # (two M=1 passes; 4 batch col-groups run concurrently on the PE)
        psU = psum.tile([128, NT], F32, tag="ps", name="psU")
        psS = psum.tile([128, NT], F32, tag="ps", name="psS")
        for j in range(3):
            for kk in range(2):
                k = 2 * j + kk
                for b in range(BPC):
                    mv = xtj[j][:, kk, b * NT:(b + 1) * NT]
                    nc.tensor.matmul(
                        psU[32 * b:32 * b + 1, :], pqw_sb[:, 2 * k:2 * k + 1], mv,
                        tile_position=(0, 32 * b),
                        start=(k == 0), stop=(k == KC - 1),
                    )
                    nc.tensor.matmul(
                        psS[32 * b:32 * b + 1, :], pqw_sb[:, 2 * k + 1:2 * k + 2], mv,
                        tile_position=(0, 32 * b),
                        start=(k == 0), stop=(k == KC - 1),
                    )

        # ---- recurrence on 128-partition tiles (live rows 32b), U/S from PSUM
        Ulab = work.tile([128, NT], F32, tag="Ulab")
        Slab = work.tile([128, NT], F32, tag="Slab")
        nc.vector.scalar_tensor_tensor(
            Ulab[:, :], psU[:, :], karB[:, 0:1], labB, AL.add, AL.mult
        )
        nc.vector.scalar_tensor_tensor(
            Slab[:, :], psS[:, :], karB[:, 1:2], labB, AL.add, AL.mult
        )
        resp = work.tile([128, NT], F32, tag="resp")
        junk = work.tile([128, NT], F32, tag="junk")
        Gt = work.tile([128, NIT], F32, tag="Gt")
        nc.vector.scalar_tensor_tensor(
            junk[:, :], Ulab[:, :], 1.0, glmB[0], AL.is_lt, AL.mult,
            accum_out=Gt[:, 0:1],
        )
        for t in range(1, NIT):
            nc.vector.scalar_tensor_tensor(
                resp[:, :], Slab[:, :], Gt[:, t - 1:t],
                Ulab[:, :] if t == 1 else resp[:, :], AL.mult, AL.add
            )
            nc.vector.scalar_tensor_tensor(
                junk[:, :], resp[:, :], float(RHO ** -t), glmB[t],
                AL.is_lt, AL.mult, accum_out=Gt[:, t:t + 1],
            )
        ctil5 = work.tile([128, 1], F32, tag="ctil5")
        nc.vector.reduce_sum(ctil5[:, :], Gt[:, :], axis=mybir.AxisListType.X)
        # tiny per-batch exports (scalar HWDGE ring; done under the xs stream)
        for b in range(BPC):
            nc.scalar.dma_start(cto[b:b + 1, :], ctil5[32 * b:32 * b + 1, :])

        # ---- search stage: per batch b, bank h rows 32b..32b+2 accumulate
        # [P, Q]; chunk 0 fp16 moving, chunks 1..5 e3m4 moving, fp16 stationary
        bank = [psum.tile([128, 512], F32, tag="ps", name=f"bank{h}")
                for h in range(2)]
        for c in range(KC):
            for h in range(2):
                for b in range(BPC):
                    if c == 0:
                        rhs = xs16_sb[:, b * NS + h * 512:b * NS + (h + 1) * 512]
                    else:
                        rhs = xs8_sb[c - 1][:, b * NS + h * 512:b * NS + (h + 1) * 512]
                    nc.tensor.matmul(
                        bank[h][32 * b:32 * b + 2, :],
                        pqw_sb[:, 12 + 2 * c:14 + 2 * c],
                        rhs,
                        tile_position=(0, 32 * b),
                        start=(c == 0),
                        stop=(c == KC - 1),
                    )

        # ---- stage live rows out of PSUM (fp16 cast) and export as one tile
        stage = work.tile([128, 2, 512], F16, tag="stage")
        nc.scalar.copy(stage[:, 0, :], bank[0][:, :])
        nc.vector.tensor_copy(stage[:, 1, :], bank[1][:, :])
        nc.sync.dma_start(pqo[:, :, :], stage[:, :, :])

    nc.finalize()
    return nc


def _host_prep(inputs):
    """Host-side precomputation from the small replicated weights."""
    mask = np.asarray(inputs["target_mask"], np.float32).reshape(B, NT)
    W = np.asarray(inputs["conv_w"], np.float64)
    cb = np.asarray(inputs["conv_b"], np.float64)
    gamma = np.asarray(inputs["bn_gamma"], np.float64)
    beta = np.asarray(inputs["bn_beta"], np.float64)
    mean = np.asarray(inputs["bn_mean"], np.float64)
    var = np.asarray(inputs["bn_var"], np.float64)
    f0 = np.asarray(inputs["filter_init"], np.float64).reshape(D)

    inv_std = gamma / np.sqrt(var + BN_EPS)
    cvec = (cb - mean) * inv_std + beta
    p = W.T @ (f0 * inv_std)
    q = W.T @ inv_std
    k1 = float(f0 @ cvec)
    k2 = float(cvec.sum())

    imp = p ** 2 + 0.05 * np.abs(p * q) + 0.0025 * q ** 2
    perm = np.argsort(-imp).astype(np.int64)
    pp, qp = p[perm], q[perm]

    pqw_h = np.zeros((128, 24), np.float16)
    pqw_h[:, 0:12:2] = p.reshape(KC, 128).T.astype(np.float16)
    pqw_h[:, 1:12:2] = q.reshape(KC, 128).T.astype(np.float16)
    pqw_h[:, 12:24:2] = pp.reshape(KC, 128).T.astype(np.float16)
    pqw_h[:, 13:24:2] = qp.reshape(KC, 128).T.astype(np.float16)

    yy, xx = np.meshgrid(
        np.arange(HT, dtype=np.float32), np.arange(WT, dtype=np.float32), indexing="ij"
    )
    yf, xf = yy.reshape(-1), xx.reshape(-1)
    msum = np.maximum(mask.sum(1), np.float32(1.0))
    cy = (mask * yf).sum(1) / msum
    cx = (mask * xf).sum(1) / msum
    d2 = (xf[None, :] - cx[:, None]) ** 2 + (yf[None, :] - cy[:, None]) ** 2
    labh = np.exp(-d2 / np.float32(2.0 * SIGMA * SIGMA)).astype(np.float32)
    glmh = (np.float32(LR / NT) * labh * mask).astype(np.float32)
    glmth = [(glmh * np.float32(RHO ** -(t + 1))).astype(np.float32)
             for t in range(NIT)]
    karr_row = np.array([k1, k2, 0.0, 0.0], np.float64).astype(np.float32)
    return perm, pqw_h, karr_row, labh, glmth, k1, k2


def postprocess(pqo, cto, k1, k2):
    """out_b = a5*(P + ctil5*Q) + a5*(k1 + k2*ctil5)."""
    r = pqo.astype(np.float64).reshape(BPC, 32, 2 * 512)[:, 0:2, :]
    P, Q = r[:, 0], r[:, 1]
    ct = cto.reshape(BPC, 1).astype(np.float64)
    o = A5 * (P + ct * Q) + A5 * k1 + A5 * k2 * ct
    return o.astype(np.float32).reshape(BPC, 1, HS, WS)


def make_in_maps(inputs):
    perm, pqw_h, karr_row, labh, glmth, k1, k2 = _host_prep(inputs)
    _CACHE["combine"] = (k1, k2)

    sf = np.asarray(inputs["search_features"], np.float32).reshape(B, D, NS)
    tf_ = np.asarray(inputs["target_features"], np.float32).reshape(B, D, NT)
    sfp = sf[:, perm, :]
    csth = np.concatenate(
        [labh] + glmth + [np.broadcast_to(karr_row[None], (B, 4))], axis=1
    ).astype(np.float32)

    in_maps = []
    for cid in range(NCORES):
        s = slice(BPC * cid, BPC * (cid + 1))
        xt_c = tf_[s].reshape(BPC, KC, 128, NT).transpose(2, 1, 0, 3)
        xt_c = np.ascontiguousarray(xt_c.reshape(128, KC, BPC * NT)).astype(NP_F8)
        xs0 = sfp[s, :KHI, :].transpose(1, 0, 2)
        xs0 = np.ascontiguousarray(xs0.reshape(128, BPC * NS)).astype(np.float16)
        xsl = sfp[s, KHI:, :].reshape(BPC, KC8, 128, NS).transpose(2, 1, 0, 3)
        xsl = np.ascontiguousarray(xsl.reshape(128, KC8, BPC * NS)).astype(NP_F8)
        in_maps.append({
            "xt8": xt_c,
            "xs16": xs0,
            "xs8": xsl,
            "pqw": pqw_h,
            "cst": np.ascontiguousarray(csth[s]),
        })
    return in_maps


def run(inputs, trace=False, **kwargs):
    if "nc" not in _CACHE:
        _CACHE["nc"] = build()
    nc = _CACHE["nc"]
    in_maps = make_in_maps(inputs)
    last_err = None
    for _attempt in range(3):
        try:
            res = run_bass_kernel_spmd(
                nc, in_maps, core_ids=list(range(NCORES)), trace=trace, **kwargs
            )
            break
        except Exception as e:  # transient NRT device faults recover on retry
            last_err = e
            time.sleep(2.0)
    else:
        raise last_err
    k1, k2 = _CACHE["combine"]
    outs = [
        postprocess(res.results[c]["pqo"], res.results[c]["cto"], k1, k2)
        for c in range(NCORES)
    ]
    return np.concatenate(outs, axis=0), res


def kernel(**inputs) -> np.ndarray:
    out, _ = run(inputs)
    return out


# revision 6
# speedup vs baseline: 1.0960x; 1.0960x over previous
"""Bass/Trainium2 kernel for nn_DiscriminativeCorrelationFilter.

Math
----
Reference computes, per batch b:
  sp = BN(W @ xs_b), tp = BN(W @ xt_b)        (1x1 conv 768->768 + eval-mode BN)
  label from mask centroid (Gaussian); f_0 = f_init; 5 iterations of a
  hinge-gradient update whose gradient is a per-batch SCALAR; then
  out_b = f_5 . sp.
Because BN(W@x) = inv_std .* (W@x) + cvec and f_t stays in
span{f_init, ones}, every channel contraction collapses onto two fixed
vectors p = W^T (f_init .* inv_std), q = W^T inv_std with scalars
k1 = f_init.cvec, k2 = sum(cvec):
    f_t . BN(W@x) = a_t (p^T x + k1) + c_t (q^T x + k2),  a_t = rho^t.
Device work = stream the features through [p;q] projections + a tiny
5-step scalar recurrence; out = a5*(P + ctil5*Q) + a5*(k1 + k2*ctil5)
rides the host unshard step (65 KFLOP total).

Performance structure (v3; the kernel is feature-DMA bound):
  * the PE multiplies fp16 stationary x fp8 moving exactly (verified
    on HW), so all stationaries are fp16 and only the feature STREAM
    is quantized.
  * search features: channels permuted by projection-weight energy;
    top 128 channels stream fp16, remaining 640 as five e3m4 chunks
    (3.67 MB/core). Search PSUM rows per batch: [P, Q] at partition
    32b via col-group packing; one fp16 tile export; host combine.
  * target features all e3m4 (0.77 MB/core), 3 chunk-pair DMAs so
    projections start on first arrival. U = p^T xt and S = q^T xt are
    two M=1 passes into col-group 32b of two PSUM tiles, so U_b and
    S_b land on the SAME partition row 32b -- the whole recurrence
    runs on 128-partition tiles (live rows 32b) with label/glm
    constants DMA'd straight to rows 32b, reading U/S from PSUM.
    No cross-partition shuffles anywhere.
  * every feature tensor is host-packed SBUF-shaped: each DMA is one
    fully contiguous multi-KB segment per partition, issued on the
    sync HWDGE ring in consumption order (xt pairs, xs16, xs8 c1..5).
End-to-end quantization error (deterministic, fixed seed): ~1.1e-2
absmax-relative vs the 2e-2 gate.

Sharding: data-parallel over batch, 4 batches per core on 8 cores.
"""

import time

import numpy as np
from contextlib import ExitStack

import concourse.bacc as bacc
import concourse.mybir as mybir
import concourse.tile as tile
from concourse.bass_utils import run_bass_kernel_spmd
import ml_dtypes

# ---------------- problem constants (hardcoded; kernel.py must be standalone)
B = 32
D = 768
HS = WS = 32
HT = WT = 16
NS = HS * WS      # 1024
NT = HT * WT      # 256
NCORES = 8
BPC = B // NCORES  # 4
KC = D // 128      # 6
KHI = 128          # channels kept in fp16 (chunk 0 after permutation)
KC8 = (D - KHI) // 128   # 5 fp8 chunks

LR = 0.1
LAM = 0.01
SIGMA = 2.0
NIT = 5
BN_EPS = 1e-5
RHO = 1.0 - LR * LAM
A5 = RHO ** NIT

F32 = mybir.dt.float32
F16 = mybir.dt.float16
F8 = mybir.dt.float8e3       # e3m4
NP_F8 = ml_dtypes.float8_e3m4

_CACHE = {}


def build():
    nc = bacc.Bacc()
    xt8 = nc.dram_tensor("xt8", (128, KC, BPC * NT), F8, kind="ExternalInput")
    xs16 = nc.dram_tensor("xs16", (128, BPC * NS), F16, kind="ExternalInput")
    xs8 = nc.dram_tensor("xs8", (128, KC8, BPC * NS), F8, kind="ExternalInput")
    # cols 0..11: natural-order [p_k, q_k] pairs (target stage)
    # cols 12..23: permuted [p_c, q_c] pairs (search stage, chunks 0..5)
    pqw = nc.dram_tensor("pqw", (128, 24), F16, kind="ExternalInput")
    cstd = nc.dram_tensor("cst", (BPC, 6 * NT + 4), F32, kind="ExternalInput")
    pqo = nc.dram_tensor("pqo", (128, 2, 512), F16, kind="ExternalOutput")
    cto = nc.dram_tensor("cto", (BPC, 1), F32, kind="ExternalOutput")

    AL = mybir.AluOpType
    CW = 6 * NT + 4

    with tile.TileContext(nc) as tc, ExitStack() as ctx:
        const = ctx.enter_context(tc.tile_pool(name="const", bufs=1))
        feats = ctx.enter_context(tc.tile_pool(name="feats", bufs=1))
        work = ctx.enter_context(tc.tile_pool(name="work", bufs=1))
        psum = ctx.enter_context(tc.tile_pool(name="psum", bufs=8, space="PSUM"))

        # ---- constants (scalar/ACT HWDGE ring): pqw first (first consumer),
        # then per-batch constants straight to partition rows 32b
        pqw_sb = const.tile([128, 24], F16, tag="pqw")
        nc.scalar.dma_start(pqw_sb[:, :], pqw[:, :])
        cstB = const.tile([128, CW], F32, tag="cstB")
        cstB_v = cstB.rearrange("(a z) f -> a z f", z=32)[:, 0:1, :]
        nc.scalar.dma_start(cstB_v, cstd.rearrange("a (o f) -> a o f", o=1))
        labB = cstB[:, 0:NT]
        glmB = [cstB[:, (1 + t) * NT:(2 + t) * NT] for t in range(NIT)]
        karB = cstB[:, 6 * NT:6 * NT + 4]

        # ---- feature loads (sync HWDGE ring) in consumption order
        xtj = []
        for j in range(3):
            t = feats.tile([128, 2, BPC * NT], F8, tag=f"xt{j}", name=f"xt{j}")
            nc.sync.dma_start(t[:, :, :], xt8[:, 2 * j:2 * j + 2, :])
            xtj.append(t)
        xs16_sb = feats.tile([128, BPC * NS], F16, tag="xs16", name="xs16")
        nc.sync.dma_start(xs16_sb[:, :], xs16[:, :])
        xs8_sb = []
        for c in range(KC8):
            t = feats.tile([128, BPC * NS], F8, tag=f"xs8_{c}", name=f"xs8_{c}")
            nc.sync.dma_start(t[:, :], xs8[:, c, :])
            xs8_sb.append(t)

        # ---- target stage: U_b -> psU row 32b, S_b -> psS row 32b
        # (two M=1 passes; 4 batch col-groups run concurrently on the PE)
        psU = psum.tile([128, NT], F32, tag="ps", name="psU")
        psS = psum.tile([128, NT], F32, tag="ps", name="psS")
        for j in range(3):
            for kk in range(2):
                k = 2 * j + kk
                for b in range(BPC):
                    mv = xtj[j][:, kk, b * NT:(b + 1) * NT]
                    nc.tensor.matmul(
                        psU[32 * b:32 * b + 1, :], pqw_sb[:, 2 * k:2 * k + 1], mv,
                        tile_position=(0, 32 * b),
                        start=(k == 0), stop=(k == KC - 1),
                    )
                    nc.tensor.matmul(
                        psS[32 * b:32 * b + 1, :], pqw_sb[:, 2 * k + 1:2 * k + 2], mv,
                        tile_position=(0, 32 * b),
                        start=(k == 0), stop=(k == KC - 1),
                    )

        # ---- recurrence on 128-partition tiles (live rows 32b), U/S from PSUM
        Ulab = work.tile([128, NT], F32, tag="Ulab")
        Slab = work.tile([128, NT], F32, tag="Slab")
        nc.vector.scalar_tensor_tensor(
            Ulab[:, :], psU[:, :], karB[:, 0:1], labB, AL.add, AL.mult
        )
        nc.vector.scalar_tensor_tensor(
            Slab[:, :], psS[:, :], karB[:, 1:2], labB, AL.add, AL.mult
        )
        resp = work.tile([128, NT], F32, tag="resp")
        junk = work.tile([128, NT], F32, tag="junk")
        Gt = work.tile([128, NIT], F32, tag="Gt")
        nc.vector.scalar_tensor_tensor(
            junk[:, :], Ulab[:, :], 1.0, glmB[0], AL.is_lt, AL.mult,
            accum_out=Gt[:, 0:1],
        )
        for t in range(1, NIT):
            nc.vector.scalar_tensor_tensor(
                resp[:, :], Slab[:, :], Gt[:, t - 1:t],
                Ulab[:, :] if t == 1 else resp[:, :], AL.mult, AL.add
            )
            nc.vector.scalar_tensor_tensor(
                junk[:, :], resp[:, :], float(RHO ** -t), glmB[t],
                AL.is_lt, AL.mult, accum_out=Gt[:, t:t + 1],
            )
        ctil5 = work.tile([128, 1], F32, tag="ctil5")
        nc.vector.reduce_sum(ctil5[:, :], Gt[:, :], axis=mybir.AxisListType.X)
        # tiny export (gpsimd SWDGE ring; done under the xs stream)
        ctil5_v = ctil5.rearrange("(a z) f -> a z f", z=32)[:, 0:1, :]
        nc.gpsimd.dma_start(cto.rearrange("a (o f) -> a o f", o=1), ctil5_v)

        # ---- search stage: per batch b, bank h rows 32b..32b+2 accumulate
        # [P, Q]; chunk 0 fp16 moving, chunks 1..5 e3m4 moving, fp16 stationary
        bank = [psum.tile([128, 512], F32, tag="ps", name=f"bank{h}")
                for h in range(2)]
        for c in range(KC):
            for h in range(2):
                for b in range(BPC):
                    if c == 0:
                        rhs = xs16_sb[:, b * NS + h * 512:b * NS + (h + 1) * 512]
                    else:
                        rhs = xs8_sb[c - 1][:, b * NS + h * 512:b * NS + (h + 1) * 512]
                    nc.tensor.matmul(
                        bank[h][32 * b:32 * b + 2, :],
                        pqw_sb[:, 12 + 2 * c:14 + 2 * c],
                        rhs,
                        tile_position=(0, 32 * b),
                        start=(c == 0),
                        stop=(c == KC - 1),
                    )

        # ---- stage live rows out of PSUM (fp16 cast) and export as one tile
        stage = work.tile([128, 2, 512], F16, tag="stage")
        nc.scalar.copy(stage[:, 0, :], bank[0][:, :])
        nc.vector.tensor_copy(stage[:, 1, :], bank[1][:, :])
        nc.sync.dma_start(pqo[:, :, :], stage[:, :, :])

    nc.finalize()
    return nc


def _host_prep(inputs):
    """Host-side precomputation from the small replicated weights."""
    mask = np.asarray(inputs["target_mask"], np.float32).reshape(B, NT)
    W = np.asarray(inputs["conv_w"], np.float64)
    cb = np.asarray(inputs["conv_b"], np.float64)
    gamma = np.asarray(inputs["bn_gamma"], np.float64)
    beta = np.asarray(inputs["bn_beta"], np.float64)
    mean = np.asarray(inputs["bn_mean"], np.float64)
    var = np.asarray(inputs["bn_var"], np.float64)
    f0 = np.asarray(inputs["filter_init"], np.float64).reshape(D)

    inv_std = gamma / np.sqrt(var + BN_EPS)
    cvec = (cb - mean) * inv_std + beta
    p = W.T @ (f0 * inv_std)
    q = W.T @ inv_std
    k1 = float(f0 @ cvec)
    k2 = float(cvec.sum())

    imp = p ** 2 + 0.05 * np.abs(p * q) + 0.0025 * q ** 2
    perm = np.argsort(-imp).astype(np.int64)
    pp, qp = p[perm], q[perm]

    pqw_h = np.zeros((128, 24), np.float16)
    pqw_h[:, 0:12:2] = p.reshape(KC, 128).T.astype(np.float16)
    pqw_h[:, 1:12:2] = q.reshape(KC, 128).T.astype(np.float16)
    pqw_h[:, 12:24:2] = pp.reshape(KC, 128).T.astype(np.float16)
    pqw_h[:, 13:24:2] = qp.reshape(KC, 128).T.astype(np.float16)

    yy, xx = np.meshgrid(
        np.arange(HT, dtype=np.float32), np.arange(WT, dtype=np.float32), indexing="ij"
    )
    yf, xf = yy.reshape(-1), xx.reshape(-1)
    msum = np.maximum(mask.sum(1), np.float32(1.0))
    cy = (mask * yf).sum(1) / msum
    cx = (mask * xf).sum(1) / msum
    d2 = (xf[None, :] - cx[:, None]) ** 2 + (yf[None, :] - cy[:, None]) ** 2
    labh = np.exp(-d2 / np.float32(2.0 * SIGMA * SIGMA)).astype(np.float32)
    glmh = (np.float32(LR / NT) * labh * mask).astype(np.float32)
    glmth = [(glmh * np.float32(RHO ** -(t + 1))).astype(np.float32)
             for t in range(NIT)]
    karr_row = np.array([k1, k2, 0.0, 0.0], np.float64).astype(np.float32)
    return perm, pqw_h, karr_row, labh, glmth, k1, k2


def postprocess(pqo, cto, k1, k2):
    """out_b = a5*(P + ctil5*Q) + a5*(k1 + k2*ctil5)."""
    r = pqo.astype(np.float64).reshape(BPC, 32, 2 * 512)[:, 0:2, :]
    P, Q = r[:, 0], r[:, 1]
    ct = cto.reshape(BPC, 1).astype(np.float64)
    o = A5 * (P + ct * Q) + A5 * k1 + A5 * k2 * ct
    return o.astype(np.float32).reshape(BPC, 1, HS, WS)


def make_in_maps(inputs):
    perm, pqw_h, karr_row, labh, glmth, k1, k2 = _host_prep(inputs)
    _CACHE["combine"] = (k1, k2)

    sf = np.asarray(inputs["search_features"], np.float32).reshape(B, D, NS)
    tf_ = np.asarray(inputs["target_features"], np.float32).reshape(B, D, NT)
    sfp = sf[:, perm, :]
    csth = np.concatenate(
        [labh] + glmth + [np.broadcast_to(karr_row[None], (B, 4))], axis=1
    ).astype(np.float32)

    in_maps = []
    for cid in range(NCORES):
        s = slice(BPC * cid, BPC * (cid + 1))
        xt_c = tf_[s].reshape(BPC, KC, 128, NT).transpose(2, 1, 0, 3)
        xt_c = np.ascontiguousarray(xt_c.reshape(128, KC, BPC * NT)).astype(NP_F8)
        xs0 = sfp[s, :KHI, :].transpose(1, 0, 2)
        xs0 = np.ascontiguousarray(xs0.reshape(128, BPC * NS)).astype(np.float16)
        xsl = sfp[s, KHI:, :].reshape(BPC, KC8, 128, NS).transpose(2, 1, 0, 3)
        xsl = np.ascontiguousarray(xsl.reshape(128, KC8, BPC * NS)).astype(NP_F8)
        in_maps.append({
            "xt8": xt_c,
            "xs16": xs0,
            "xs8": xsl,
            "pqw": pqw_h,
            "cst": np.ascontiguousarray(csth[s]),
        })
    return in_maps


def run(inputs, trace=False, **kwargs):
    if "nc" not in _CACHE:
        _CACHE["nc"] = build()
    nc = _CACHE["nc"]
    in_maps = make_in_maps(inputs)
    last_err = None
    for _attempt in range(3):
        try:
            res = run_bass_kernel_spmd(
                nc, in_maps, core_ids=list(range(NCORES)), trace=trace, **kwargs
            )
            break
        except Exception as e:  # transient NRT device faults recover on retry
            last_err = e
            time.sleep(2.0)
    else:
        raise last_err
    k1, k2 = _CACHE["combine"]
    outs = [
        postprocess(res.results[c]["pqo"], res.results[c]["cto"], k1, k2)
        for c in range(NCORES)
    ]
    return np.concatenate(outs, axis=0), res


def kernel(**inputs) -> np.ndarray:
    out, _ = run(inputs)
    return out


# revision 11
# speedup vs baseline: 1.1921x; 1.0877x over previous
"""Bass/Trainium2 kernel for nn_DiscriminativeCorrelationFilter.

Math
----
Reference computes, per batch b:
  sp = BN(W @ xs_b), tp = BN(W @ xt_b)        (1x1 conv 768->768 + eval-mode BN)
  label from mask centroid (Gaussian); f_0 = f_init; 5 iterations of a
  hinge-gradient update whose gradient is a per-batch SCALAR; then
  out_b = f_5 . sp.
Because BN(W@x) = inv_std .* (W@x) + cvec and f_t stays in
span{f_init, ones}, every channel contraction collapses onto two fixed
vectors p = W^T (f_init .* inv_std), q = W^T inv_std with scalars
k1 = f_init.cvec, k2 = sum(cvec):
    f_t . BN(W@x) = a_t (p^T x + k1) + c_t (q^T x + k2),  a_t = rho^t.
Device work = stream the features through [p;q] projections + a tiny
5-step scalar recurrence; out = a5*(P + ctil5*Q) + a5*(k1 + k2*ctil5)
rides the host unshard step (65 KFLOP total).

Performance structure (v5; the kernel is feature-DMA bound):
  * the PE multiplies fp16 stationary x fp8 moving exactly (verified
    on HW), so the stationaries stay fp16 and only the feature STREAM
    is quantized: ALL features in e3m4, pre-scaled by sqrt(2) (the
    scale shifts the binade boundaries to a lower-error spot for this
    data; it is folded into the fp16 stationaries, p/s and q/s, so
    the math is unchanged). 3.92 MB/core total stream.
  * search: per batch b, PSUM rows [P, Q] at partition 32b via
    col-group packing; compact 16 KB fancy-AP export of the 8 live
    rows; host does the 3-term combine.
  * target: 3 chunk-pair DMAs first in the stream; U = (p/s)^T xt'
    and S = (q/s)^T xt' as two M=1 passes into col-group 32b of two
    PSUM tiles, so U_b and S_b land on the SAME partition row 32b --
    the recurrence runs on 128-partition tiles (live rows 32b) with
    label/glm constants DMA'd straight to rows 32b via one
    partition-strided AP, reading U/S from PSUM. No cross-partition
    shuffles anywhere.
  * every feature tensor is host-packed SBUF-shaped: each DMA is one
    fully contiguous multi-KB segment per partition, issued on the
    sync HWDGE ring in consumption order.
End-to-end quantization error (deterministic, fixed seed): ~1.4e-2
absmax-relative vs the 2e-2 gate.

Sharding: data-parallel over batch, 4 batches per core on 8 cores.
"""

import time

import numpy as np
from contextlib import ExitStack

import concourse.bacc as bacc
import concourse.mybir as mybir
import concourse.tile as tile
from concourse.bass_utils import run_bass_kernel_spmd
import ml_dtypes

# ---------------- problem constants (hardcoded; kernel.py must be standalone)
B = 32
D = 768
HS = WS = 32
HT = WT = 16
NS = HS * WS      # 1024
NT = HT * WT      # 256
NCORES = 8
BPC = B // NCORES  # 4
KC = D // 128      # 6

LR = 0.1
LAM = 0.01
SIGMA = 2.0
NIT = 5
BN_EPS = 1e-5
RHO = 1.0 - LR * LAM
A5 = RHO ** NIT
SCL = float(np.sqrt(2.0))    # feature pre-scale (folded into stationaries)

F32 = mybir.dt.float32
F16 = mybir.dt.float16
F8 = mybir.dt.float8e3       # e3m4
NP_F8 = ml_dtypes.float8_e3m4

_CACHE = {}


def build():
    nc = bacc.Bacc()
    xt8 = nc.dram_tensor("xt8", (128, KC, BPC * NT), F8, kind="ExternalInput")
    xs8 = nc.dram_tensor("xs8", (128, KC, BPC * NS), F8, kind="ExternalInput")
    pqw = nc.dram_tensor("pqw", (128, 2 * KC), F16, kind="ExternalInput")
    cstd = nc.dram_tensor("cst", (BPC, 6 * NT + 4), F32, kind="ExternalInput")
    pqo = nc.dram_tensor("pqo", (128, 2, 512), F16, kind="ExternalOutput")
    cto = nc.dram_tensor("cto", (BPC, 1), F32, kind="ExternalOutput")

    AL = mybir.AluOpType
    CW = 6 * NT + 4

    with tile.TileContext(nc) as tc, ExitStack() as ctx:
        const = ctx.enter_context(tc.tile_pool(name="const", bufs=1))
        feats = ctx.enter_context(tc.tile_pool(name="feats", bufs=1))
        work = ctx.enter_context(tc.tile_pool(name="work", bufs=1))
        psum = ctx.enter_context(tc.tile_pool(name="psum", bufs=8, space="PSUM"))

        # ---- constants (scalar/ACT HWDGE ring): pqw, then per-batch
        # constants straight to partition rows 32b via one strided AP
        pqw_sb = const.tile([128, 2 * KC], F16, tag="pqw")
        nc.scalar.dma_start(pqw_sb[:, :], pqw[:, :])
        cstB = const.tile([128, CW], F32, tag="cstB")
        cstB_v = cstB.rearrange("(a z) f -> a z f", z=32)[:, 0:1, :]
        nc.scalar.dma_start(cstB_v, cstd.rearrange("a (o f) -> a o f", o=1))
        labB = cstB[:, 0:NT]
        glmB = [cstB[:, (1 + t) * NT:(2 + t) * NT] for t in range(NIT)]
        karB = cstB[:, 6 * NT:6 * NT + 4]

        # ---- feature loads (sync HWDGE ring) in consumption order
        xtj = []
        for j in range(3):
            t = feats.tile([128, 2, BPC * NT], F8, tag=f"xt{j}", name=f"xt{j}")
            nc.sync.dma_start(t[:, :, :], xt8[:, 2 * j:2 * j + 2, :])
            xtj.append(t)
        xs_sb = []
        for c in range(KC):
            t = feats.tile([128, BPC * NS], F8, tag=f"xs{c}", name=f"xs{c}")
            nc.sync.dma_start(t[:, :], xs8[:, c, :])
            xs_sb.append(t)

        # ---- target stage: U_b -> psU row 32b, S_b -> psS row 32b
        # (two M=1 passes; 4 batch col-groups share the PE)
        psU = psum.tile([128, NT], F32, tag="ps", name="psU")
        psS = psum.tile([128, NT], F32, tag="ps", name="psS")
        for j in range(3):
            for kk in range(2):
                k = 2 * j + kk
                for b in range(BPC):
                    mv = xtj[j][:, kk, b * NT:(b + 1) * NT]
                    nc.tensor.matmul(
                        psU[32 * b:32 * b + 1, :], pqw_sb[:, 2 * k:2 * k + 1], mv,
                        tile_position=(0, 32 * b),
                        start=(k == 0), stop=(k == KC - 1),
                    )
                    nc.tensor.matmul(
                        psS[32 * b:32 * b + 1, :], pqw_sb[:, 2 * k + 1:2 * k + 2], mv,
                        tile_position=(0, 32 * b),
                        start=(k == 0), stop=(k == KC - 1),
                    )

        # ---- recurrence on 128-partition tiles (live rows 32b), U/S from PSUM
        Ulab = work.tile([128, NT], F32, tag="Ulab")
        Slab = work.tile([128, NT], F32, tag="Slab")
        nc.vector.scalar_tensor_tensor(
            Ulab[:, :], psU[:, :], karB[:, 0:1], labB, AL.add, AL.mult
        )
        nc.vector.scalar_tensor_tensor(
            Slab[:, :], psS[:, :], karB[:, 1:2], labB, AL.add, AL.mult
        )
        resp = work.tile([128, NT], F32, tag="resp")
        junk = work.tile([128, NT], F32, tag="junk")
        Gt = work.tile([128, NIT], F32, tag="Gt")
        nc.vector.scalar_tensor_tensor(
            junk[:, :], Ulab[:, :], 1.0, glmB[0], AL.is_lt, AL.mult,
            accum_out=Gt[:, 0:1],
        )
        for t in range(1, NIT):
            nc.vector.scalar_tensor_tensor(
                resp[:, :], Slab[:, :], Gt[:, t - 1:t],
                Ulab[:, :] if t == 1 else resp[:, :], AL.mult, AL.add
            )
            nc.vector.scalar_tensor_tensor(
                junk[:, :], resp[:, :], float(RHO ** -t), glmB[t],
                AL.is_lt, AL.mult, accum_out=Gt[:, t:t + 1],
            )
        ctil5 = work.tile([128, 1], F32, tag="ctil5")
        nc.vector.reduce_sum(ctil5[:, :], Gt[:, :], axis=mybir.AxisListType.X)
        # tiny export (scalar HWDGE ring; lands well before the stage copy)
        ctil5_v = ctil5.rearrange("(a z) f -> a z f", z=32)[:, 0:1, :]
        nc.scalar.dma_start(cto.rearrange("a (o f) -> a o f", o=1), ctil5_v)

        # ---- search stage: per batch b, bank h rows 32b..32b+2 accumulate
        # [P, Q]; e3m4 moving, fp16 stationary
        bank = [psum.tile([128, 512], F32, tag="ps", name=f"bank{h}")
                for h in range(2)]
        for c in range(KC):
            for h in range(2):
                for b in range(BPC):
                    rhs = xs_sb[c][:, b * NS + h * 512:b * NS + (h + 1) * 512]
                    nc.tensor.matmul(
                        bank[h][32 * b:32 * b + 2, :],
                        pqw_sb[:, 2 * c:2 * c + 2],
                        rhs,
                        tile_position=(0, 32 * b),
                        start=(c == 0),
                        stop=(c == KC - 1),
                    )

        # ---- stage live rows out of PSUM (fp16 cast), compact 16 KB export
        stage = work.tile([128, 2, 512], F16, tag="stage")
        nc.scalar.copy(stage[:, 0, :], bank[0][:, :])
        nc.vector.tensor_copy(stage[:, 1, :], bank[1][:, :])
        nc.sync.dma_start(pqo[:, :, :], stage[:, :, :])

    nc.finalize()
    return nc


def _host_prep(inputs):
    """Host-side precomputation from the small replicated weights."""
    mask = np.asarray(inputs["target_mask"], np.float32).reshape(B, NT)
    W = np.asarray(inputs["conv_w"], np.float64)
    cb = np.asarray(inputs["conv_b"], np.float64)
    gamma = np.asarray(inputs["bn_gamma"], np.float64)
    beta = np.asarray(inputs["bn_beta"], np.float64)
    mean = np.asarray(inputs["bn_mean"], np.float64)
    var = np.asarray(inputs["bn_var"], np.float64)
    f0 = np.asarray(inputs["filter_init"], np.float64).reshape(D)

    inv_std = gamma / np.sqrt(var + BN_EPS)
    cvec = (cb - mean) * inv_std + beta
    p = W.T @ (f0 * inv_std)
    q = W.T @ inv_std
    k1 = float(f0 @ cvec)
    k2 = float(cvec.sum())

    # stationaries carry the 1/SCL de-scale of the e3m4 feature pre-scale
    pqw_h = np.zeros((128, 2 * KC), np.float16)
    pqw_h[:, 0::2] = (p / SCL).reshape(KC, 128).T.astype(np.float16)
    pqw_h[:, 1::2] = (q / SCL).reshape(KC, 128).T.astype(np.float16)

    yy, xx = np.meshgrid(
        np.arange(HT, dtype=np.float32), np.arange(WT, dtype=np.float32), indexing="ij"
    )
    yf, xf = yy.reshape(-1), xx.reshape(-1)
    msum = np.maximum(mask.sum(1), np.float32(1.0))
    cy = (mask * yf).sum(1) / msum
    cx = (mask * xf).sum(1) / msum
    d2 = (xf[None, :] - cx[:, None]) ** 2 + (yf[None, :] - cy[:, None]) ** 2
    labh = np.exp(-d2 / np.float32(2.0 * SIGMA * SIGMA)).astype(np.float32)
    glmh = (np.float32(LR / NT) * labh * mask).astype(np.float32)
    glmth = [(glmh * np.float32(RHO ** -(t + 1))).astype(np.float32)
             for t in range(NIT)]
    karr_row = np.array([k1, k2, 0.0, 0.0], np.float64).astype(np.float32)
    return pqw_h, karr_row, labh, glmth, k1, k2


def postprocess(pqo, cto, k1, k2):
    """out_b = a5*(P + ctil5*Q) + a5*(k1 + k2*ctil5)."""
    r = pqo.astype(np.float64).reshape(BPC, 32, 2 * 512)[:, 0:2, :]
    P, Q = r[:, 0], r[:, 1]
    ct = cto.reshape(BPC, 1).astype(np.float64)
    o = A5 * (P + ct * Q) + A5 * k1 + A5 * k2 * ct
    return o.astype(np.float32).reshape(BPC, 1, HS, WS)


def make_in_maps(inputs):
    pqw_h, karr_row, labh, glmth, k1, k2 = _host_prep(inputs)
    _CACHE["combine"] = (k1, k2)

    sf = np.asarray(inputs["search_features"], np.float32).reshape(B, D, NS)
    tf_ = np.asarray(inputs["target_features"], np.float32).reshape(B, D, NT)
    scl = np.float32(SCL)
    csth = np.concatenate(
        [labh] + glmth + [np.broadcast_to(karr_row[None], (B, 4))], axis=1
    ).astype(np.float32)

    in_maps = []
    for cid in range(NCORES):
        s = slice(BPC * cid, BPC * (cid + 1))
        xt_c = (tf_[s] * scl).reshape(BPC, KC, 128, NT).transpose(2, 1, 0, 3)
        xt_c = np.ascontiguousarray(xt_c.reshape(128, KC, BPC * NT)).astype(NP_F8)
        xs_c = (sf[s] * scl).reshape(BPC, KC, 128, NS).transpose(2, 1, 0, 3)
        xs_c = np.ascontiguousarray(xs_c.reshape(128, KC, BPC * NS)).astype(NP_F8)
        in_maps.append({
            "xt8": xt_c,
            "xs8": xs_c,
            "pqw": pqw_h,
            "cst": np.ascontiguousarray(csth[s]),
        })
    return in_maps


def run(inputs, trace=False, **kwargs):
    if "nc" not in _CACHE:
        _CACHE["nc"] = build()
    nc = _CACHE["nc"]
    in_maps = make_in_maps(inputs)
    last_err = None
    for _attempt in range(3):
        try:
            res = run_bass_kernel_spmd(
                nc, in_maps, core_ids=list(range(NCORES)), trace=trace, **kwargs
            )
            break
        except Exception as e:  # transient NRT device faults recover on retry
            last_err = e
            time.sleep(2.0)
    else:
        raise last_err
    k1, k2 = _CACHE["combine"]
    outs = [
        postprocess(res.results[c]["pqo"], res.results[c]["cto"], k1, k2)
        for c in range(NCORES)
    ]
    return np.concatenate(outs, axis=0), res


def kernel(**inputs) -> np.ndarray:
    out, _ = run(inputs)
    return out


# revision 14
# speedup vs baseline: 1.2285x; 1.0306x over previous
"""Bass/Trainium2 kernel for nn_DiscriminativeCorrelationFilter.

Math
----
Reference computes, per batch b:
  sp = BN(W @ xs_b), tp = BN(W @ xt_b)        (1x1 conv 768->768 + eval-mode BN)
  label from mask centroid (Gaussian); f_0 = f_init; 5 iterations of a
  hinge-gradient update whose gradient is a per-batch SCALAR; then
  out_b = f_5 . sp.
Because BN(W@x) = inv_std .* (W@x) + cvec and f_t stays in
span{f_init, ones}, every channel contraction collapses onto two fixed
vectors p = W^T (f_init .* inv_std), q = W^T inv_std with scalars
k1 = f_init.cvec, k2 = sum(cvec):
    f_t . BN(W@x) = a_t (p^T x + k1) + c_t (q^T x + k2),  a_t = rho^t.
Device work = stream the features through [p;q] projections + a tiny
5-step scalar recurrence; out = a5*(P + ctil5*Q) + a5*(k1 + k2*ctil5)
rides the host unshard step (65 KFLOP total).

Performance structure (v5; the kernel is feature-DMA bound):
  * the PE multiplies fp16 stationary x fp8 moving exactly (verified
    on HW), so the stationaries stay fp16 and only the feature STREAM
    is quantized: ALL features in e3m4, pre-scaled by sqrt(2) (the
    scale shifts the binade boundaries to a lower-error spot for this
    data; it is folded into the fp16 stationaries, p/s and q/s, so
    the math is unchanged). 3.92 MB/core total stream.
  * search: per batch b, PSUM rows [P, Q] at partition 32b via
    col-group packing; compact 16 KB fancy-AP export of the 8 live
    rows; host does the 3-term combine.
  * target: 3 chunk-pair DMAs first in the stream; U = (p/s)^T xt'
    and S = (q/s)^T xt' as two M=1 passes into col-group 32b of two
    PSUM tiles, so U_b and S_b land on the SAME partition row 32b --
    the recurrence runs on 128-partition tiles (live rows 32b) with
    label/glm constants DMA'd straight to rows 32b via one
    partition-strided AP, reading U/S from PSUM. No cross-partition
    shuffles anywhere.
  * every feature tensor is host-packed SBUF-shaped: each DMA is one
    fully contiguous multi-KB segment per partition, issued on the
    sync HWDGE ring in consumption order.
End-to-end quantization error (deterministic, fixed seed): ~1.4e-2
absmax-relative vs the 2e-2 gate.

Sharding: data-parallel over batch, 4 batches per core on 8 cores.
"""

import time

import numpy as np
from contextlib import ExitStack

import concourse.bacc as bacc
import concourse.mybir as mybir
import concourse.tile as tile
from concourse.bass_utils import run_bass_kernel_spmd
import ml_dtypes

# ---------------- problem constants (hardcoded; kernel.py must be standalone)
B = 32
D = 768
HS = WS = 32
HT = WT = 16
NS = HS * WS      # 1024
NT = HT * WT      # 256
NCORES = 8
BPC = B // NCORES  # 4
KC = D // 128      # 6

LR = 0.1
LAM = 0.01
SIGMA = 2.0
NIT = 5
BN_EPS = 1e-5
RHO = 1.0 - LR * LAM
A5 = RHO ** NIT
SCL = float(np.sqrt(2.0))    # feature pre-scale (folded into stationaries)

F32 = mybir.dt.float32
F16 = mybir.dt.float16
F8 = mybir.dt.float8e3       # e3m4
NP_F8 = ml_dtypes.float8_e3m4

_CACHE = {}


def build():
    nc = bacc.Bacc()
    xt8 = nc.dram_tensor("xt8", (128, KC, BPC * NT), F8, kind="ExternalInput")
    xs8 = nc.dram_tensor("xs8", (128, KC, BPC * NS), F8, kind="ExternalInput")
    pqw = nc.dram_tensor("pqw", (128, 2 * KC), F16, kind="ExternalInput")
    cstd = nc.dram_tensor("cst", (BPC, 6 * NT + 4), F32, kind="ExternalInput")
    pqo = nc.dram_tensor("pqo", (128, 2, 512), F16, kind="ExternalOutput")
    cto = nc.dram_tensor("cto", (BPC, 1), F32, kind="ExternalOutput")

    AL = mybir.AluOpType
    CW = 6 * NT + 4

    with tile.TileContext(nc) as tc, ExitStack() as ctx:
        const = ctx.enter_context(tc.tile_pool(name="const", bufs=1))
        feats = ctx.enter_context(tc.tile_pool(name="feats", bufs=1))
        work = ctx.enter_context(tc.tile_pool(name="work", bufs=1))
        psum = ctx.enter_context(tc.tile_pool(name="psum", bufs=8, space="PSUM"))

        # ---- PE warm-up: ~3.5us of dummy matmuls while the feature DMAs are
        # still in flight releases the HAM clock throttle (cold PE runs at
        # 1.2 GHz; every later matmul would pay 2x otherwise)
        wmov = const.tile([128, 512], F8, tag="wmov")
        nc.gpsimd.memset(wmov[:, :], 0.0)
        wstat = const.tile([128, 2], F16, tag="wstat")
        nc.vector.memset(wstat[:, :], 0.0)
        psW = psum.tile([2, 512], F32, tag="ps", name="psW")
        for _ in range(8):
            nc.tensor.matmul(psW[:, :], wstat[:, :], wmov[:, :],
                             start=True, stop=True)

        # ---- constants (scalar/ACT HWDGE ring): pqw, then per-batch
        # constants straight to partition rows 32b via one strided AP
        pqw_sb = const.tile([128, 2 * KC], F16, tag="pqw")
        nc.scalar.dma_start(pqw_sb[:, :], pqw[:, :])
        cstB = const.tile([128, CW], F32, tag="cstB")
        cstB_v = cstB.rearrange("(a z) f -> a z f", z=32)[:, 0:1, :]
        nc.scalar.dma_start(cstB_v, cstd.rearrange("a (o f) -> a o f", o=1))
        labB = cstB[:, 0:NT]
        glmB = [cstB[:, (1 + t) * NT:(2 + t) * NT] for t in range(NIT)]
        karB = cstB[:, 6 * NT:6 * NT + 4]

        # ---- feature loads (sync HWDGE ring) in consumption order
        xt_sb = feats.tile([128, KC, BPC * NT], F8, tag="xt", name="xt")
        nc.sync.dma_start(xt_sb[:, :, :], xt8[:, :, :])
        xs_sb = []
        for c in range(KC):
            t = feats.tile([128, BPC * NS], F8, tag=f"xs{c}", name=f"xs{c}")
            nc.sync.dma_start(t[:, :], xs8[:, c, :])
            xs_sb.append(t)

        # ---- target stage: U_b -> psU row 32b, S_b -> psS row 32b
        # (two M=1 passes; 4 batch col-groups share the PE)
        psU = psum.tile([128, NT], F32, tag="ps", name="psU")
        psS = psum.tile([128, NT], F32, tag="ps", name="psS")
        for j in range(3):
            for kk in range(2):
                k = 2 * j + kk
                for b in range(BPC):
                    mv = xt_sb[:, k, b * NT:(b + 1) * NT]
                    nc.tensor.matmul(
                        psU[32 * b:32 * b + 1, :], pqw_sb[:, 2 * k:2 * k + 1], mv,
                        tile_position=(0, 32 * b),
                        start=(k == 0), stop=(k == KC - 1),
                    )
                    nc.tensor.matmul(
                        psS[32 * b:32 * b + 1, :], pqw_sb[:, 2 * k + 1:2 * k + 2], mv,
                        tile_position=(0, 32 * b),
                        start=(k == 0), stop=(k == KC - 1),
                    )

        # ---- recurrence on 128-partition tiles (live rows 32b), U/S from PSUM
        Ulab = work.tile([128, NT], F32, tag="Ulab")
        Slab = work.tile([128, NT], F32, tag="Slab")
        nc.vector.scalar_tensor_tensor(
            Ulab[:, :], psU[:, :], karB[:, 0:1], labB, AL.add, AL.mult
        )
        nc.vector.scalar_tensor_tensor(
            Slab[:, :], psS[:, :], karB[:, 1:2], labB, AL.add, AL.mult
        )
        resp = work.tile([128, NT], F32, tag="resp")
        junk = work.tile([128, NT], F32, tag="junk")
        Gt = work.tile([128, NIT], F32, tag="Gt")
        nc.vector.scalar_tensor_tensor(
            junk[:, :], Ulab[:, :], 1.0, glmB[0], AL.is_lt, AL.mult,
            accum_out=Gt[:, 0:1],
        )
        for t in range(1, NIT):
            nc.vector.scalar_tensor_tensor(
                resp[:, :], Slab[:, :], Gt[:, t - 1:t],
                Ulab[:, :] if t == 1 else resp[:, :], AL.mult, AL.add
            )
            nc.vector.scalar_tensor_tensor(
                junk[:, :], resp[:, :], float(RHO ** -t), glmB[t],
                AL.is_lt, AL.mult, accum_out=Gt[:, t:t + 1],
            )
        ctil5 = work.tile([128, 1], F32, tag="ctil5")
        nc.vector.reduce_sum(ctil5[:, :], Gt[:, :], axis=mybir.AxisListType.X)
        # tiny export (scalar HWDGE ring; lands well before the stage copy)
        ctil5_v = ctil5.rearrange("(a z) f -> a z f", z=32)[:, 0:1, :]
        nc.scalar.dma_start(cto.rearrange("a (o f) -> a o f", o=1), ctil5_v)

        # ---- search stage: per batch b, bank h rows 32b..32b+2 accumulate
        # [P, Q]; e3m4 moving, fp16 stationary
        bank = [psum.tile([128, 512], F32, tag="ps", name=f"bank{h}")
                for h in range(2)]
        for c in range(KC):
            for h in range(2):
                for b in range(BPC):
                    rhs = xs_sb[c][:, b * NS + h * 512:b * NS + (h + 1) * 512]
                    nc.tensor.matmul(
                        bank[h][32 * b:32 * b + 2, :],
                        pqw_sb[:, 2 * c:2 * c + 2],
                        rhs,
                        tile_position=(0, 32 * b),
                        start=(c == 0),
                        stop=(c == KC - 1),
                    )

        # ---- stage live rows out of PSUM (fp16 cast), compact 16 KB export
        stage = work.tile([128, 2, 512], F16, tag="stage")
        nc.scalar.copy(stage[:, 0, :], bank[0][:, :])
        nc.vector.tensor_copy(stage[:, 1, :], bank[1][:, :])
        nc.sync.dma_start(pqo[:, :, :], stage[:, :, :])

    nc.finalize()
    return nc


def _host_prep(inputs):
    """Host-side precomputation from the small replicated weights."""
    mask = np.asarray(inputs["target_mask"], np.float32).reshape(B, NT)
    W = np.asarray(inputs["conv_w"], np.float64)
    cb = np.asarray(inputs["conv_b"], np.float64)
    gamma = np.asarray(inputs["bn_gamma"], np.float64)
    beta = np.asarray(inputs["bn_beta"], np.float64)
    mean = np.asarray(inputs["bn_mean"], np.float64)
    var = np.asarray(inputs["bn_var"], np.float64)
    f0 = np.asarray(inputs["filter_init"], np.float64).reshape(D)

    inv_std = gamma / np.sqrt(var + BN_EPS)
    cvec = (cb - mean) * inv_std + beta
    p = W.T @ (f0 * inv_std)
    q = W.T @ inv_std
    k1 = float(f0 @ cvec)
    k2 = float(cvec.sum())

    # stationaries carry the 1/SCL de-scale of the e3m4 feature pre-scale
    pqw_h = np.zeros((128, 2 * KC), np.float16)
    pqw_h[:, 0::2] = (p / SCL).reshape(KC, 128).T.astype(np.float16)
    pqw_h[:, 1::2] = (q / SCL).reshape(KC, 128).T.astype(np.float16)

    yy, xx = np.meshgrid(
        np.arange(HT, dtype=np.float32), np.arange(WT, dtype=np.float32), indexing="ij"
    )
    yf, xf = yy.reshape(-1), xx.reshape(-1)
    msum = np.maximum(mask.sum(1), np.float32(1.0))
    cy = (mask * yf).sum(1) / msum
    cx = (mask * xf).sum(1) / msum
    d2 = (xf[None, :] - cx[:, None]) ** 2 + (yf[None, :] - cy[:, None]) ** 2
    labh = np.exp(-d2 / np.float32(2.0 * SIGMA * SIGMA)).astype(np.float32)
    glmh = (np.float32(LR / NT) * labh * mask).astype(np.float32)
    glmth = [(glmh * np.float32(RHO ** -(t + 1))).astype(np.float32)
             for t in range(NIT)]
    karr_row = np.array([k1, k2, 0.0, 0.0], np.float64).astype(np.float32)
    return pqw_h, karr_row, labh, glmth, k1, k2


def postprocess(pqo, cto, k1, k2):
    """out_b = a5*(P + ctil5*Q) + a5*(k1 + k2*ctil5)."""
    r = pqo.astype(np.float64).reshape(BPC, 32, 2 * 512)[:, 0:2, :]
    P, Q = r[:, 0], r[:, 1]
    ct = cto.reshape(BPC, 1).astype(np.float64)
    o = A5 * (P + ct * Q) + A5 * k1 + A5 * k2 * ct
    return o.astype(np.float32).reshape(BPC, 1, HS, WS)


def make_in_maps(inputs):
    pqw_h, karr_row, labh, glmth, k1, k2 = _host_prep(inputs)
    _CACHE["combine"] = (k1, k2)

    sf = np.asarray(inputs["search_features"], np.float32).reshape(B, D, NS)
    tf_ = np.asarray(inputs["target_features"], np.float32).reshape(B, D, NT)
    scl = np.float32(SCL)
    csth = np.concatenate(
        [labh] + glmth + [np.broadcast_to(karr_row[None], (B, 4))], axis=1
    ).astype(np.float32)

    in_maps = []
    for cid in range(NCORES):
        s = slice(BPC * cid, BPC * (cid + 1))
        xt_c = (tf_[s] * scl).reshape(BPC, KC, 128, NT).transpose(2, 1, 0, 3)
        xt_c = np.ascontiguousarray(xt_c.reshape(128, KC, BPC * NT)).astype(NP_F8)
        xs_c = (sf[s] * scl).reshape(BPC, KC, 128, NS).transpose(2, 1, 0, 3)
        xs_c = np.ascontiguousarray(xs_c.reshape(128, KC, BPC * NS)).astype(NP_F8)
        in_maps.append({
            "xt8": xt_c,
            "xs8": xs_c,
            "pqw": pqw_h,
            "cst": np.ascontiguousarray(csth[s]),
        })
    return in_maps


def run(inputs, trace=False, **kwargs):
    if "nc" not in _CACHE:
        _CACHE["nc"] = build()
    nc = _CACHE["nc"]
    in_maps = make_in_maps(inputs)
    last_err = None
    for _attempt in range(3):
        try:
            res = run_bass_kernel_spmd(
                nc, in_maps, core_ids=list(range(NCORES)), trace=trace, **kwargs
            )
            break
        except Exception as e:  # transient NRT device faults recover on retry
            last_err = e
            time.sleep(2.0)
    else:
        raise last_err
    k1, k2 = _CACHE["combine"]
    outs = [
        postprocess(res.results[c]["pqo"], res.results[c]["cto"], k1, k2)
        for c in range(NCORES)
    ]
    return np.concatenate(outs, axis=0), res


def kernel(**inputs) -> np.ndarray:
    out, _ = run(inputs)
    return out


# revision 18
# speedup vs baseline: 1.2980x; 1.0565x over previous
"""Bass/Trainium2 kernel for nn_DiscriminativeCorrelationFilter.

Math
----
Reference computes, per batch b:
  sp = BN(W @ xs_b), tp = BN(W @ xt_b)        (1x1 conv 768->768 + eval-mode BN)
  label from mask centroid (Gaussian); f_0 = f_init; 5 iterations of a
  hinge-gradient update whose gradient is a per-batch SCALAR; then
  out_b = f_5 . sp.
Because BN(W@x) = inv_std .* (W@x) + cvec and f_t stays in
span{f_init, ones}, every channel contraction collapses onto two fixed
vectors p = W^T (f_init .* inv_std), q = W^T inv_std with scalars
k1 = f_init.cvec, k2 = sum(cvec):
    f_t . BN(W@x) = a_t (p^T x + k1) + c_t (q^T x + k2),  a_t = rho^t.
Device work = stream the features through [p;q] projections + a tiny
5-step scalar recurrence; out = a5*(P + ctil5*Q) + a5*(k1 + k2*ctil5)
rides the host unshard step (65 KFLOP total).

Performance structure (v5; the kernel is feature-DMA bound):
  * the PE multiplies fp16 stationary x fp8 moving exactly (verified
    on HW), so the stationaries stay fp16 and only the feature STREAM
    is quantized: ALL features in e3m4, pre-scaled by sqrt(2) (the
    scale shifts the binade boundaries to a lower-error spot for this
    data; it is folded into the fp16 stationaries, p/s and q/s, so
    the math is unchanged). 3.92 MB/core total stream.
  * search: per batch b, PSUM rows [P, Q] at partition 32b via
    col-group packing; compact 16 KB fancy-AP export of the 8 live
    rows; host does the 3-term combine.
  * target: 3 chunk-pair DMAs first in the stream; U = (p/s)^T xt'
    and S = (q/s)^T xt' as two M=1 passes into col-group 32b of two
    PSUM tiles, so U_b and S_b land on the SAME partition row 32b --
    the recurrence runs on 128-partition tiles (live rows 32b) with
    label/glm constants DMA'd straight to rows 32b via one
    partition-strided AP, reading U/S from PSUM. No cross-partition
    shuffles anywhere.
  * every feature tensor is host-packed SBUF-shaped: each DMA is one
    fully contiguous multi-KB segment per partition, issued on the
    sync HWDGE ring in consumption order.
End-to-end quantization error (deterministic, fixed seed): ~1.4e-2
absmax-relative vs the 2e-2 gate.

Sharding: data-parallel over batch, 4 batches per core on 8 cores.
"""

import time

import numpy as np
from contextlib import ExitStack

import concourse.bacc as bacc
import concourse.mybir as mybir
import concourse.tile as tile
from concourse.bass_utils import run_bass_kernel_spmd
import ml_dtypes

# ---------------- problem constants (hardcoded; kernel.py must be standalone)
B = 32
D = 768
HS = WS = 32
HT = WT = 16
NS = HS * WS      # 1024
NT = HT * WT      # 256
NCORES = 8
BPC = B // NCORES  # 4
KC = D // 128      # 6

LR = 0.1
LAM = 0.01
SIGMA = 2.0
NIT = 5
BN_EPS = 1e-5
RHO = 1.0 - LR * LAM
A5 = RHO ** NIT
SCL = float(np.sqrt(2.0))    # feature pre-scale (folded into stationaries)

F32 = mybir.dt.float32
F16 = mybir.dt.float16
F8 = mybir.dt.float8e3       # e3m4
NP_F8 = ml_dtypes.float8_e3m4

_CACHE = {}


def build():
    nc = bacc.Bacc()
    xt8 = nc.dram_tensor("xt8", (128, KC, BPC * NT), F8, kind="ExternalInput")
    xs8 = nc.dram_tensor("xs8", (128, KC, BPC * NS), F8, kind="ExternalInput")
    pqw = nc.dram_tensor("pqw", (128, 2 * KC), F16, kind="ExternalInput")
    cstd = nc.dram_tensor("cst", (BPC, 6 * NT + 4), F32, kind="ExternalInput")
    pqo = nc.dram_tensor("pqo", (128, 2, 512), F16, kind="ExternalOutput")
    cto = nc.dram_tensor("cto", (BPC, 1), F32, kind="ExternalOutput")

    AL = mybir.AluOpType
    CW = 6 * NT + 4

    with tile.TileContext(nc) as tc, ExitStack() as ctx:
        const = ctx.enter_context(tc.tile_pool(name="const", bufs=1))
        feats = ctx.enter_context(tc.tile_pool(name="feats", bufs=1))
        work = ctx.enter_context(tc.tile_pool(name="work", bufs=1))
        psum = ctx.enter_context(tc.tile_pool(name="psum", bufs=8, space="PSUM"))

        # ---- PE warm-up: ~3.5us of dummy matmuls while the feature DMAs are
        # still in flight releases the HAM clock throttle (cold PE runs at
        # 1.2 GHz; every later matmul would pay 2x otherwise)
        wmov = const.tile([128, 512], F8, tag="wmov")
        nc.gpsimd.memset(wmov[:, :], 0.0)
        wstat = const.tile([128, 2], F16, tag="wstat")
        nc.vector.memset(wstat[:, :], 0.0)
        psW = psum.tile([2, 512], F32, tag="ps", name="psW")
        for _ in range(8):
            nc.tensor.matmul(psW[:, :], wstat[:, :], wmov[:, :],
                             start=True, stop=True)

        # ---- constants (scalar/ACT HWDGE ring): pqw, then per-batch
        # constants straight to partition rows 32b via one strided AP
        pqw_sb = const.tile([128, 2 * KC], F16, tag="pqw")
        nc.scalar.dma_start(pqw_sb[:, :], pqw[:, :])
        cstB = const.tile([128, CW], F32, tag="cstB")
        cstB_v = cstB.rearrange("(a z) f -> a z f", z=32)[:, 0:1, :]
        nc.scalar.dma_start(cstB_v, cstd.rearrange("a (o f) -> a o f", o=1))
        labB = cstB[:, 0:NT]
        glmB = [cstB[:, (1 + t) * NT:(2 + t) * NT] for t in range(NIT)]
        karB = cstB[:, 6 * NT:6 * NT + 4]

        # ---- feature loads (sync HWDGE ring) in consumption order
        xt_sb = feats.tile([128, KC, BPC * NT], F8, tag="xt", name="xt")
        nc.sync.dma_start(xt_sb[:, :, :], xt8[:, :, :])
        xs_sb = []
        for c in range(KC - 1):
            t = feats.tile([128, BPC * NS], F8, tag=f"xs{c}", name=f"xs{c}")
            nc.sync.dma_start(t[:, :], xs8[:, c, :])
            xs_sb.append(t)
        # last chunk arrives h-major in two halves: bank0's accumulation can
        # finish and stage while bank1's half is still streaming
        xsl_sb = []
        for h in range(2):
            t = feats.tile([128, BPC * 512], F8, tag=f"xsl{h}", name=f"xsl{h}")
            nc.sync.dma_start(t[:, :], xs8[:, KC - 1, h * BPC * 512:(h + 1) * BPC * 512])
            xsl_sb.append(t)

        # ---- target stage: U_b -> psU row 32b, S_b -> psS row 32b
        # (two M=1 passes; 4 batch col-groups share the PE)
        psU = psum.tile([128, NT], F32, tag="ps", name="psU")
        psS = psum.tile([128, NT], F32, tag="ps", name="psS")
        for j in range(3):
            for kk in range(2):
                k = 2 * j + kk
                for b in range(BPC):
                    mv = xt_sb[:, k, b * NT:(b + 1) * NT]
                    nc.tensor.matmul(
                        psU[32 * b:32 * b + 1, :], pqw_sb[:, 2 * k:2 * k + 1], mv,
                        tile_position=(0, 32 * b),
                        start=(k == 0), stop=(k == KC - 1),
                    )
                    nc.tensor.matmul(
                        psS[32 * b:32 * b + 1, :], pqw_sb[:, 2 * k + 1:2 * k + 2], mv,
                        tile_position=(0, 32 * b),
                        start=(k == 0), stop=(k == KC - 1),
                    )

        # ---- recurrence on 128-partition tiles (live rows 32b), U/S from PSUM
        Ulab = work.tile([128, NT], F32, tag="Ulab")
        Slab = work.tile([128, NT], F32, tag="Slab")
        nc.vector.scalar_tensor_tensor(
            Ulab[:, :], psU[:, :], karB[:, 0:1], labB, AL.add, AL.mult
        )
        nc.vector.scalar_tensor_tensor(
            Slab[:, :], psS[:, :], karB[:, 1:2], labB, AL.add, AL.mult
        )
        resp = work.tile([128, NT], F32, tag="resp")
        junk = work.tile([128, NT], F32, tag="junk")
        Gt = work.tile([128, NIT], F32, tag="Gt")
        nc.vector.scalar_tensor_tensor(
            junk[:, :], Ulab[:, :], 1.0, glmB[0], AL.is_lt, AL.mult,
            accum_out=Gt[:, 0:1],
        )
        for t in range(1, NIT):
            nc.vector.scalar_tensor_tensor(
                resp[:, :], Slab[:, :], Gt[:, t - 1:t],
                Ulab[:, :] if t == 1 else resp[:, :], AL.mult, AL.add
            )
            nc.vector.scalar_tensor_tensor(
                junk[:, :], resp[:, :], float(RHO ** -t), glmB[t],
                AL.is_lt, AL.mult, accum_out=Gt[:, t:t + 1],
            )
        ctil5 = work.tile([128, 1], F32, tag="ctil5")
        nc.vector.reduce_sum(ctil5[:, :], Gt[:, :], axis=mybir.AxisListType.X)
        # tiny export (scalar HWDGE ring; lands well before the stage copy)
        ctil5_v = ctil5.rearrange("(a z) f -> a z f", z=32)[:, 0:1, :]
        nc.scalar.dma_start(cto.rearrange("a (o f) -> a o f", o=1), ctil5_v)

        # ---- search stage: per batch b, bank h rows 32b..32b+2 accumulate
        # [P, Q]; e3m4 moving, fp16 stationary
        bank = [psum.tile([128, 512], F32, tag="ps", name=f"bank{h}")
                for h in range(2)]
        for c in range(KC):
            for h in range(2):
                for b in range(BPC):
                    if c < KC - 1:
                        rhs = xs_sb[c][:, b * NS + h * 512:b * NS + (h + 1) * 512]
                    else:
                        rhs = xsl_sb[h][:, b * 512:(b + 1) * 512]
                    nc.tensor.matmul(
                        bank[h][32 * b:32 * b + 2, :],
                        pqw_sb[:, 2 * c:2 * c + 2],
                        rhs,
                        tile_position=(0, 32 * b),
                        start=(c == 0),
                        stop=(c == KC - 1),
                    )

        # ---- stage live rows out of PSUM (fp16 cast), per-bank exports so
        # bank0's transfer overlaps bank1's copy
        stage = work.tile([128, 2, 512], F16, tag="stage")
        nc.scalar.copy(stage[:, 0, :], bank[0][:, :])
        nc.sync.dma_start(pqo[:, 0, :], stage[:, 0, :])
        nc.vector.tensor_copy(stage[:, 1, :], bank[1][:, :])
        nc.sync.dma_start(pqo[:, 1, :], stage[:, 1, :])

    nc.finalize()
    return nc


def _host_prep(inputs):
    """Host-side precomputation from the small replicated weights."""
    mask = np.asarray(inputs["target_mask"], np.float32).reshape(B, NT)
    W = np.asarray(inputs["conv_w"], np.float64)
    cb = np.asarray(inputs["conv_b"], np.float64)
    gamma = np.asarray(inputs["bn_gamma"], np.float64)
    beta = np.asarray(inputs["bn_beta"], np.float64)
    mean = np.asarray(inputs["bn_mean"], np.float64)
    var = np.asarray(inputs["bn_var"], np.float64)
    f0 = np.asarray(inputs["filter_init"], np.float64).reshape(D)

    inv_std = gamma / np.sqrt(var + BN_EPS)
    cvec = (cb - mean) * inv_std + beta
    p = W.T @ (f0 * inv_std)
    q = W.T @ inv_std
    k1 = float(f0 @ cvec)
    k2 = float(cvec.sum())

    # stationaries carry the 1/SCL de-scale of the e3m4 feature pre-scale
    pqw_h = np.zeros((128, 2 * KC), np.float16)
    pqw_h[:, 0::2] = (p / SCL).reshape(KC, 128).T.astype(np.float16)
    pqw_h[:, 1::2] = (q / SCL).reshape(KC, 128).T.astype(np.float16)

    yy, xx = np.meshgrid(
        np.arange(HT, dtype=np.float32), np.arange(WT, dtype=np.float32), indexing="ij"
    )
    yf, xf = yy.reshape(-1), xx.reshape(-1)
    msum = np.maximum(mask.sum(1), np.float32(1.0))
    cy = (mask * yf).sum(1) / msum
    cx = (mask * xf).sum(1) / msum
    d2 = (xf[None, :] - cx[:, None]) ** 2 + (yf[None, :] - cy[:, None]) ** 2
    labh = np.exp(-d2 / np.float32(2.0 * SIGMA * SIGMA)).astype(np.float32)
    glmh = (np.float32(LR / NT) * labh * mask).astype(np.float32)
    glmth = [(glmh * np.float32(RHO ** -(t + 1))).astype(np.float32)
             for t in range(NIT)]
    karr_row = np.array([k1, k2, 0.0, 0.0], np.float64).astype(np.float32)
    return pqw_h, karr_row, labh, glmth, k1, k2


def postprocess(pqo, cto, k1, k2):
    """out_b = a5*(P + ctil5*Q) + a5*(k1 + k2*ctil5)."""
    r = pqo.astype(np.float64).reshape(BPC, 32, 2 * 512)[:, 0:2, :]
    P, Q = r[:, 0], r[:, 1]
    ct = cto.reshape(BPC, 1).astype(np.float64)
    o = A5 * (P + ct * Q) + A5 * k1 + A5 * k2 * ct
    return o.astype(np.float32).reshape(BPC, 1, HS, WS)


def make_in_maps(inputs):
    pqw_h, karr_row, labh, glmth, k1, k2 = _host_prep(inputs)
    _CACHE["combine"] = (k1, k2)

    sf = np.asarray(inputs["search_features"], np.float32).reshape(B, D, NS)
    tf_ = np.asarray(inputs["target_features"], np.float32).reshape(B, D, NT)
    scl = np.float32(SCL)
    csth = np.concatenate(
        [labh] + glmth + [np.broadcast_to(karr_row[None], (B, 4))], axis=1
    ).astype(np.float32)

    in_maps = []
    for cid in range(NCORES):
        s = slice(BPC * cid, BPC * (cid + 1))
        xt_c = (tf_[s] * scl).reshape(BPC, KC, 128, NT).transpose(2, 1, 0, 3)
        xt_c = np.ascontiguousarray(xt_c.reshape(128, KC, BPC * NT)).astype(NP_F8)
        xs_c = (sf[s] * scl).reshape(BPC, KC, 128, NS).transpose(2, 1, 0, 3)
        # last chunk h-major: [h][b][512] so it can stream as two half-DMAs
        xs_c = xs_c.copy()
        xs_c[:, KC - 1] = np.ascontiguousarray(
            xs_c[:, KC - 1].reshape(128, BPC, 2, 512).transpose(0, 2, 1, 3)
        ).reshape(128, BPC, NS)
        xs_c = np.ascontiguousarray(xs_c.reshape(128, KC, BPC * NS)).astype(NP_F8)
        in_maps.append({
            "xt8": xt_c,
            "xs8": xs_c,
            "pqw": pqw_h,
            "cst": np.ascontiguousarray(csth[s]),
        })
    return in_maps


def run(inputs, trace=False, **kwargs):
    if "nc" not in _CACHE:
        _CACHE["nc"] = build()
    nc = _CACHE["nc"]
    in_maps = make_in_maps(inputs)
    last_err = None
    for _attempt in range(3):
        try:
            res = run_bass_kernel_spmd(
                nc, in_maps, core_ids=list(range(NCORES)), trace=trace, **kwargs
            )
            break
        except Exception as e:  # transient NRT device faults recover on retry
            last_err = e
            time.sleep(2.0)
    else:
        raise last_err
    k1, k2 = _CACHE["combine"]
    outs = [
        postprocess(res.results[c]["pqo"], res.results[c]["cto"], k1, k2)
        for c in range(NCORES)
    ]
    return np.concatenate(outs, axis=0), res


def kernel(**inputs) -> np.ndarray:
    out, _ = run(inputs)
    return out
